# revision 7
# baseline (speedup 1.0000x reference)
"""NemotronH Mamba2 decoder layer on 8 Trainium2 cores (Bass/Tile), v2.

Sharding: tensor-parallel over the 8 SSM groups (1 group = 8 heads / core),
data-parallel over tokens for the norm and the output projection.

v2 changes vs v1:
  - prologue: 64-token-block ownership; pass A in 4 subtiles with 4
    pipelined AllGathers, so in_proj group g waits only on AG_g
  - single ACT table (natural_log_exp): SiLUs batched per group, z staged
    via table-free copies, conv bias folded into the DVE MAC chain
  - decay-row broadcasts (crow/erow/dtot) moved from PE ones-matmuls to
    gpsimd partition_broadcast, prefetched one chunk ahead
  - out_proj: even-half n0/n1 passes interleaved into group-3 chunk slots;
    w_out streamed as [128,512] column-quarter sets with deadline ordering
  - xbca / dt rows kept as rolling half-size buffers (SBUF headroom)
"""
import os
import sys
import types

import numpy as np
import ml_dtypes

# --- axon NTFF profile hook shim (lets trace=True work in this container) ---
try:
    import antenv
    if "antenv.axon_hooks" not in sys.modules:
        try:
            from trn_agent_boot.trn_boot import _ntff_profile_via_ctypes
            _hooks = types.ModuleType("antenv.axon_hooks")
            _hook = _ntff_profile_via_ctypes("/opt/axon/libaxon_pjrt.so")
            _hooks.get_axon_ntff_profile_hook = lambda: _hook
            sys.modules["antenv.axon_hooks"] = _hooks
            antenv.axon_hooks = _hooks
        except Exception:
            pass
except Exception:
    pass

import concourse.bass as bass  # noqa: F401
import concourse.bacc as bacc
import concourse.tile as tile
import concourse.mybir as mybir
import concourse.bass_utils as bass_utils

bass_utils.upload_artifacts = lambda tmpdir: tmpdir  # no S3 in-container

# Restrict the ACT-table chooser to the two tables this kernel needs
# (exp+ln+square+copy live together in natural_log_exp_and_others; silu
# only in silu_and_others). The default chooser picks the first table
# containing a function, causing a 1.3us reload at every exp<->ln
# boundary. Emptying the other sets (order and ids preserved) makes both
# exp and ln resolve to the combined table.
import concourse.bacc as _bacc_mod

_orig_get_tables = _bacc_mod.get_activation_tables
_KEEP_TABLES = ("natural_log_exp_and_others", "silu_and_others")


def _patched_get_tables(arch):
    tabs = _orig_get_tables(arch)
    return {name: (fns if name in _KEEP_TABLES else set())
            for name, fns in tabs.items()}


_bacc_mod.get_activation_tables = _patched_get_tables

FP32 = mybir.dt.float32
BF16 = mybir.dt.bfloat16
AF = mybir.ActivationFunctionType
ALU = mybir.AluOpType

NCORES = 8
BT = 2048        # B*L tokens
DM = 2048        # model dim
TPC = 256        # tokens per core (data-parallel slice)
NH = 8           # heads per core
PD = 64          # head dim
Q = 128          # scan chunk length
NCH = BT // Q    # 16 chunks
NGRP = 4         # token groups for in_proj pipelining
GSZ = BT // NGRP # 512
EPS = 1e-5
CVC = 518        # conv buffer cols: 3 history + 512 + 3 slack

_BUILT = None
LAST_RESULTS = None


def _build():
    nc = bacc.Bacc("TRN2", target_bir_lowering=False, debug=False,
                   num_devices=NCORES)

    def inp(name, shape, dt):
        return nc.dram_tensor(name, shape, dt, kind="ExternalInput").ap()

    # hid+res precomputed on host: full tokens 0-1023 (replicated groups
    # 0/1) and the core's own 128 tokens for AllGather slots 2/3
    hr = inp("hr", [2 * GSZ, DM], BF16)
    hso = inp("hso", [128, DM], BF16)
    w_in_t = inp("w_in_t", [DM, 1288], BF16)
    w_out_t = inp("w_out_t", [4096, DM], BF16)
    a_col = inp("a_col", [NH, 1], FP32)
    dtb_col = inp("dtb_col", [NH, 1], FP32)
    dp_col = inp("dp_col", [128, 4], FP32)
    convw = inp("convw", [128, 24], FP32)
    convb = inp("convb", [128, 6], FP32)
    ones_col_bf = inp("ones_col_bf", [128, 1], BF16)
    m0_bf = inp("m0_bf", [128, 128], BF16)   # [s,t]: -1e30 where s>t else 0
    i_bf = inp("i_bf", [128, 128], BF16)
    i_f32 = inp("i_f32", [128, 128], FP32)

    out_s = nc.dram_tensor("out_s", [TPC, DM], FP32,
                           kind="ExternalOutput").ap()

    rg = [list(range(NCORES))]

    with tile.TileContext(nc) as tc:
        with (
            tc.tile_pool(name="const", bufs=1) as cpool,
            tc.tile_pool(name="dram", bufs=1, space="DRAM") as dram,
        ):
            fin = tc.alloc_tile_pool(name="fin", bufs=1)
            mid = tc.alloc_tile_pool(name="mid", bufs=1)
            wpool = tc.alloc_tile_pool(name="wpool", bufs=1, side="right")
            # ---------------- constants ----------------
            c_ones_col = cpool.tile([128, 1], BF16)
            nc.scalar.dma_start(c_ones_col[:], ones_col_bf[:])
            c_m0 = cpool.tile([128, 128], BF16)
            nc.scalar.dma_start(c_m0[:], m0_bf[:])
            c_ibf = cpool.tile([128, 128], BF16)
            nc.scalar.dma_start(c_ibf[:], i_bf[:])
            c_if32 = cpool.tile([128, 128], FP32)
            nc.scalar.dma_start(c_if32[:], i_f32[:])
            c_acol = cpool.tile([NH, 1], FP32)
            nc.scalar.dma_start(c_acol[:], a_col[:])
            c_dtb = cpool.tile([NH, 1], FP32)
            nc.scalar.dma_start(c_dtb[:], dtb_col[:])
            c_dp = cpool.tile([128, 4], FP32)
            nc.scalar.dma_start(c_dp[:], dp_col[:])
            c_convw = cpool.tile([128, 24], FP32)
            nc.scalar.dma_start(c_convw[:], convw[:])
            c_convb = cpool.tile([128, 6], FP32)
            nc.scalar.dma_start(c_convb[:], convb[:])
            z8 = cpool.tile([NH, Q], FP32)
            nc.vector.memset(z8[:], 0.0)
            c_eps = cpool.tile([128, 1], FP32)
            nc.vector.memset(c_eps[:], EPS)
            c_one = cpool.tile([128, 1], FP32)
            nc.vector.memset(c_one[:], 1.0)

            wt = [wpool.tile([128, 1288], BF16, name=f"wt{k}")
                  for k in range(16)]

            ag_x = [dram.tile([64, DM], BF16, name=f"agx{j}")
                    for j in range(2, 4)]
            xs01 = dram.tile([2 * GSZ, DM], BF16)
            xs_d = [dram.tile([GSZ, DM], BF16, name=f"xsd{j}",
                              addr_space="Shared") for j in range(2, 4)]
            a2aE_in = dram.tile([4096, 128], BF16)
            a2aE_out = dram.tile([4096, 128], BF16)
            a2aO_in = dram.tile([4096, 128], BF16)
            a2aO_out = dram.tile([4096, 128], BF16)

            # ---------------- mid-life activations (rolling, 2 groups) -----
            xbca_all = mid.tile([128, 6 * 2 * GSZ], BF16)
            xbca = [xbca_all[:, i * 2 * GSZ:(i + 1) * 2 * GSZ]
                    for i in range(6)]
            dt_raw = mid.tile([NH, 2 * GSZ], FP32)
            ldt = mid.tile([NH, 2 * GSZ], FP32)
            a_row = mid.tile([NH, 2 * GSZ], FP32)

            # ------- pass A ------------------------------------------------
            # own tokens, slots 2/3 first: their xs feeds the two AllGathers
            # (first-collective latency ~100us is hidden behind groups 0/1,
            # which every core computes replicated from the full rows)
            with (
                tc.tile_pool(name="pal", bufs=10) as pal,
                tc.tile_pool(name="pac", bufs=2) as pac,
            ):
                def pa_load(kind, idx):
                    h = pal.tile([128, DM], BF16, tag="h")
                    if kind == "own":
                        nc.scalar.dma_start(h[0:64, :],
                                            hso[(idx - 2) * 64:
                                                (idx - 1) * 64, :])
                    else:
                        rows = slice(idx * 128, (idx + 1) * 128)
                        ring = nc.sync if idx < 4 else nc.gpsimd
                        ring.dma_start(h[:], hr[rows, :])
                    return h

                def pa_comp(kind, idx, hsum):
                    p = 64 if kind == "own" else 128
                    h = hsum[0:p, :]
                    sq = pac.tile([128, DM], FP32, tag="sq")
                    ss = pac.tile([128, 1], FP32, tag="ss")
                    nc.scalar.activation(sq[0:p, :], h, AF.Square,
                                         accum_out=ss[0:p, :])
                    ln = pac.tile([128, 1], FP32, tag="ln")
                    nc.scalar.activation(ln[0:p, :], ss[0:p, :], AF.Ln,
                                         scale=1.0 / DM, bias=c_eps[0:p, :])
                    rsq = pac.tile([128, 1], FP32, tag="rsq")
                    nc.scalar.activation(rsq[0:p, :], ln[0:p, :], AF.Exp,
                                         scale=-0.5)
                    xsb = pac.tile([128, DM], BF16, tag="xs")
                    if kind == "rep" and idx < 4:
                        nc.vector.tensor_scalar_mul(xsb[0:p, :], h,
                                                    rsq[0:p, :])
                    else:
                        nc.scalar.mul(xsb[0:p, :], h, rsq[0:p, :])
                    if kind == "own":
                        nc.scalar.dma_start(ag_x[idx - 2][:], xsb[0:64, :])
                        nc.gpsimd.collective_compute(
                            "AllGather", ALU.bypass, replica_groups=rg,
                            ins=[ag_x[idx - 2].opt()],
                            outs=[xs_d[idx - 2].opt()])
                    else:
                        rows = slice(idx * 128, (idx + 1) * 128)
                        ring = nc.sync if idx < 4 else nc.gpsimd
                        ring.dma_start(xs01[rows, :], xsb[0:p, :])

                # issue every load up front (rings are disjoint FIFOs),
                # then compute in priority order: group-0 tiles first so
                # xs01 rows 0-511 (and with them in_proj group 0) unblock
                # as early as possible
                load_order = [("rep", 0), ("rep", 4), ("own", 2),
                              ("rep", 1), ("rep", 5), ("own", 3),
                              ("rep", 2), ("rep", 6), ("rep", 3),
                              ("rep", 7)]
                comp_order = [("rep", 0), ("own", 2), ("rep", 1),
                              ("rep", 2), ("rep", 3), ("rep", 4),
                              ("own", 3), ("rep", 5), ("rep", 6),
                              ("rep", 7)]
                loaded = {}
                for item in load_order:
                    loaded[item] = pa_load(*item)
                # w_in prefetch split across both HWDGE rings, queued right
                # behind the pass-A loads so the first m-tiles aren't gated
                # on a single 5.3MB stream
                for k in range(16):
                    ring = nc.scalar if k % 2 == 0 else nc.sync
                    ring.dma_start(wt[k][:],
                                   w_in_t[k * 128:(k + 1) * 128, :])
                for item in comp_order:
                    pa_comp(item[0], item[1], loaded[item])



            # ------- interleaved in_proj + conv + dt-prep + scan -------------
            ip = tc.alloc_tile_pool(name="ip", bufs=2, side="right")
            ippsum = tc.alloc_tile_pool(name="ippsum", bufs=2, space="PSUM",
                                        side="right")
            sc = tc.alloc_tile_pool(name="sc", bufs=2)
            scst = tc.alloc_tile_pool(name="scst", bufs=2)
            bc = tc.alloc_tile_pool(name="bc", bufs=3)
            convp = tc.alloc_tile_pool(name="convp", bufs=1)
            cvt = tc.alloc_tile_pool(name="cvt", bufs=1)
            with (
                tc.tile_pool(name="ps_misc", bufs=1, space="PSUM") as ps_misc,
                tc.tile_pool(name="ps_trp", bufs=1, space="PSUM") as ps_trp,
                tc.tile_pool(name="ps_big", bufs=1, space="PSUM") as ps_big,
                tc.tile_pool(name="ps_ys", bufs=2, space="PSUM") as ps_ys,
            ):
                cvb = [convp.tile([128, CVC], BF16, name=f"cvb{i}")
                       for i in range(6)]
                for i in range(6):
                    nc.vector.memset(cvb[i][:, 0:3], 0.0)

                xt_of = {}
                sz_of = {}
                head_b_of = {}

                def emit_ip_loads(g):
                    src = (xs01[g * GSZ:(g + 1) * GSZ, :] if g < 2
                           else xs_d[g - 2][:])
                    xt = [ip.tile([128, GSZ], BF16, tag=f"xt{k}",
                                  name=f"xt{g}_{k}") for k in range(16)]
                    for k in range(16):
                        nc.sync.dma_start_transpose(
                            xt[k][:], src[:, k * 128:(k + 1) * 128])
                    xt_of[g] = xt

                def emit_ip_mtile(g, m):
                    xt = xt_of[g]
                    gc = slice((g % 2) * GSZ, (g % 2 + 1) * GSZ)
                    mrows = 8 if m == 10 else 128
                    ps = ippsum.tile([128, GSZ], FP32, tag="ipps")
                    for k in range(16):
                        nc.tensor.matmul(
                            ps[0:mrows, :],
                            wt[k][:, m * 128:m * 128 + mrows],
                            xt[k][:],
                            start=(k == 0), stop=(k == 15))
                    if m < 6:
                        nc.scalar.copy(cvb[m][:, 3:3 + GSZ], ps[:, :])
                    elif m < 10:
                        if m == 6:
                            sz_of[g] = scst.tile([128, 4 * GSZ], BF16,
                                                 tag="szall",
                                                 name=f"sz{g}")
                        szt = sz_of[g][:, (m - 6) * GSZ:(m - 5) * GSZ]
                        nc.scalar.copy(szt, ps[:, :])
                    else:
                        nc.scalar.copy(dt_raw[:, gc], ps[0:8, :])

                def emit_conv(g, lo, hi):
                    gc = slice((g % 2) * GSZ, (g % 2 + 1) * GSZ)
                    for i in range(lo, hi):
                        cw = [c_convw[:, i * 4 + k:i * 4 + k + 1]
                              for k in range(4)]
                        t0 = cvt.tile([128, GSZ], BF16, tag="cv0")
                        nc.vector.tensor_scalar(t0[:], cvb[i][:, 0:GSZ],
                                                cw[0], c_convb[:, i:i + 1],
                                                ALU.mult, ALU.add)
                        t1 = cvt.tile([128, GSZ], BF16, tag="cv1")
                        nc.vector.scalar_tensor_tensor(
                            t1[:], cvb[i][:, 1:1 + GSZ], cw[1], t0[:],
                            ALU.mult, ALU.add)
                        t2 = cvt.tile([128, GSZ], BF16, tag="cv0")
                        nc.vector.scalar_tensor_tensor(
                            t2[:], cvb[i][:, 2:2 + GSZ], cw[2], t1[:],
                            ALU.mult, ALU.add)
                        nc.vector.scalar_tensor_tensor(
                            xbca[i][:, gc], cvb[i][:, 3:3 + GSZ], cw[3],
                            t2[:], ALU.mult, ALU.add)
                        # roll conv history (zero across the batch boundary)
                        if g == 1:
                            nc.vector.memset(cvb[i][:, 0:3], 0.0)
                        else:
                            nc.vector.tensor_copy(cvb[i][:, 0:3],
                                                  cvb[i][:, GSZ:GSZ + 3])

                def emit_dtprep(g):
                    gc = slice((g % 2) * GSZ, (g % 2 + 1) * GSZ)
                    e1 = sc.tile([NH, GSZ], FP32, tag="e1")
                    nc.scalar.activation(e1[:], dt_raw[:, gc], AF.Exp,
                                         bias=c_dtb[:])
                    dtv = sc.tile([NH, GSZ], FP32, tag="dtv")
                    nc.scalar.activation(dtv[:], e1[:], AF.Ln,
                                         bias=c_one[0:NH, :])
                    nc.scalar.activation(ldt[:, gc], dtv[:], AF.Ln)
                    nc.vector.tensor_scalar_mul(a_row[:, gc], dtv[:],
                                                c_acol[:])

                def emit_silu_x(g):
                    # single strided silu op so the scheduler cannot scatter
                    # it between exp ops (table thrash)
                    xap = (xbca_all[:]
                           .rearrange("p (i c) -> p i c", i=6)
                           [:, :, (g % 2) * GSZ:(g % 2 + 1) * GSZ])
                    nc.scalar.activation(xap, xap, AF.Silu)

                def emit_silu_z(g):
                    szt = sz_of[g]
                    nc.scalar.activation(szt[:], szt[:], AF.Silu)

                def emit_head_b(ci):
                    cols = slice((ci // 4 % 2) * GSZ + (ci % 4) * Q,
                                 (ci // 4 % 2) * GSZ + (ci % 4 + 1) * Q)
                    first = (ci % 8 == 0)
                    c_t = sc.tile([NH, Q], FP32, tag="c", bufs=3,
                                  name=f"ct{ci}")
                    nc.vector.tensor_tensor_scan(
                        c_t[:], a_row[:, cols], z8[:], 0.0, ALU.add, ALU.add)
                    lc = sc.tile([NH, Q], FP32, tag="lc", bufs=3,
                                 name=f"lc{ci}")
                    nc.vector.tensor_sub(lc[:], ldt[:, cols], c_t[:])
                    wrow = sc.tile([NH, Q], FP32, tag="wrow", bufs=3,
                                   name=f"wr{ci}")
                    nc.scalar.activation(wrow[:], lc[:], AF.Exp,
                                         bias=c_t[:, Q - 1:Q])
                    crow = sc.tile([1, NH * Q], FP32, tag="crow", bufs=3,
                                   name=f"cr{ci}")
                    nc.sync.dma_start(crow[:], c_t[:])
                    crow_b = bc.tile([128, NH * Q], FP32, tag="crb",
                                     name=f"crb{ci}")
                    nc.gpsimd.partition_broadcast(crow_b[:], crow[:])
                    erow_b = None
                    if not first:
                        ecr = sc.tile([NH, Q], BF16, tag="ecr", bufs=3,
                                      name=f"ec{ci}")
                        nc.scalar.activation(ecr[:], c_t[:], AF.Exp)
                        erow = sc.tile([1, NH * Q], BF16, tag="erow",
                                       bufs=3, name=f"er{ci}")
                        nc.sync.dma_start(erow[:], ecr[:])
                        erow_b = bc.tile([128, NH * Q], BF16, tag="erb",
                                         name=f"erb{ci}")
                        nc.gpsimd.partition_broadcast(erow_b[:], erow[:])

                    # transposes: lcT cols 0:8, wrowT 8:16, c_endT 16:24
                    misc = ps_misc.tile([128, 32], FP32, tag="misc",
                                        name=f"msc{ci}")
                    nc.tensor.transpose(misc[:, 0:8], lc[:],
                                        c_if32[0:8, 0:8])
                    nc.tensor.transpose(misc[:, 8:16], wrow[:],
                                        c_if32[0:8, 0:8])
                    lwt = sc.tile([128, 16], FP32, tag="lwt", bufs=3,
                                  name=f"lwt{ci}")
                    nc.scalar.copy(lwt[:], misc[:, 0:16])
                    dtot = None
                    if not first:
                        nc.tensor.transpose(misc[0:1, 16:24],
                                            c_t[:, Q - 1:Q],
                                            c_if32[0:8, 0:8])
                        dtr = sc.tile([1, 8], FP32, tag="dtr", bufs=3,
                                      name=f"dtr{ci}")
                        nc.scalar.activation(dtr[:], misc[0:1, 16:24],
                                             AF.Exp)
                        dtot = bc.tile([128, 8], FP32, tag="dtot",
                                       name=f"dtot{ci}")
                        nc.gpsimd.partition_broadcast(dtot[:], dtr[:])

                    head_b_of[ci] = (crow_b, erow_b, lwt, dtot)

                st = {"sb": None, "sf": None}

                def emit_chunk(ci):
                    cols = slice((ci // 4 % 2) * GSZ + (ci % 4) * Q,
                                 (ci // 4 % 2) * GSZ + (ci % 4 + 1) * Q)
                    first = (ci % 8 == 0)
                    crow_b, erow_b, lwt, dtot = head_b_of.pop(ci)
                    lct = lwt[:, 0:8]

                    # Gmat [s,t] (shared by all heads of the group), masked
                    # causal via 0/1 upper-tri so dpair needs no -inf term
                    gmp = ps_big.tile([128, Q], FP32, tag="big")
                    nc.tensor.matmul(gmp[:], xbca[4][:, cols],
                                     xbca[5][:, cols], start=True, stop=True)
                    gm = sc.tile([128, Q], BF16, tag="gm")
                    nc.vector.tensor_tensor(gm[:], gmp[:], c_m0[:], ALU.mult)

                    # Cec[r] = C_fm * exp(c_r[t]) rows (state-passing only)
                    cec = None
                    if not first:
                        cec = sc.tile([128, NH * Q], BF16, tag="cec")
                        for r in range(8):
                            nc.vector.tensor_mul(
                                cec[:, r * Q:(r + 1) * Q], xbca[5][:, cols],
                                erow_b[:, r * Q:(r + 1) * Q])

                    # B token-major
                    btp = ps_trp.tile([128, Q], BF16, tag="trp")
                    nc.tensor.transpose(btp[:], xbca[4][:, cols], c_ibf[:])
                    btk = sc.tile([128, Q], BF16, tag="btk")
                    nc.scalar.copy(btk[:], btp[:])

                    s_sb_new = scst.tile([128, 512], FP32, tag="ssb")
                    s_bf_new = scst.tile([128, 512], BF16, tag="sbf")
                    s_sb_prev, s_bf_prev = st["sb"], st["sf"]

                    vch = [sc.tile([128, Q], BF16, tag=f"vch{p}",
                                   name=f"vp{ci}_{p}") for p in range(4)]

                    for pi in range(4):
                        prows = slice(pi * 128, (pi + 1) * 128)
                        # decay matrix for the head pair; mask lives in gm,
                        # clamp bounds exp() in the masked (s>t) region
                        dpair = sc.tile([128, 256], FP32, tag="dpair")
                        for hh in range(2):
                            r = pi * 2 + hh
                            sl = slice(hh * Q, (hh + 1) * Q)
                            nc.vector.tensor_scalar(
                                dpair[:, sl],
                                crow_b[:, r * Q:(r + 1) * Q],
                                lct[:, r:r + 1], 8.0,
                                ALU.add, ALU.min)
                        dexp = sc.tile([128, 256], BF16, tag="dexp")
                        nc.scalar.activation(dexp[:], dpair[:], AF.Exp)
                        mtp = sc.tile([128, 256], BF16, tag="mtp")
                        for hh in range(2):
                            sl = slice(hh * Q, (hh + 1) * Q)
                            nc.vector.tensor_mul(mtp[:, sl], dexp[:, sl],
                                                 gm[:])

                        # X token-major (pair)
                        xpp = ps_trp.tile([128, Q], BF16, tag="trp")
                        nc.tensor.transpose(xpp[:], xbca[pi][:, cols],
                                            c_ibf[:])
                        xtk = sc.tile([128, Q], BF16, tag="xtk")
                        nc.scalar.copy(xtk[:], xpp[:])
                        xw = sc.tile([128, Q], BF16, tag="xw")
                        for hh in range(2):
                            r = pi * 2 + hh
                            psl = slice(hh * PD, (hh + 1) * PD)
                            nc.vector.tensor_scalar_mul(
                                xw[:, psl], xtk[:, psl], lwt[:, 8 + r:9 + r])

                        # Y (cols 0:Q) and state outer product (cols Q:2Q)
                        ys = ps_ys.tile([128, 2 * Q], FP32, tag="ys")
                        yp = ys[:, 0:Q]
                        sp = ys[:, Q:2 * Q]
                        for hh in range(2):
                            r = pi * 2 + hh
                            orow = slice(hh * PD, (hh + 1) * PD)
                            nc.tensor.matmul(
                                yp[orow, :], xtk[:, orow],
                                mtp[:, hh * Q:(hh + 1) * Q],
                                start=True, stop=first)
                            if not first:
                                nc.tensor.matmul(
                                    yp[orow, :],
                                    s_bf_prev[:, r * PD:(r + 1) * PD],
                                    cec[:, r * Q:(r + 1) * Q],
                                    start=False, stop=True)

                        # state update
                        nc.tensor.matmul(sp[:], btk[:], xw[:], start=True,
                                         stop=True)
                        if first:
                            nc.vector.tensor_copy(s_sb_new[:, prows], sp[:])
                        else:
                            for hh in range(2):
                                r = pi * 2 + hh
                                esl = slice(r * PD, (r + 1) * PD)
                                nc.vector.scalar_tensor_tensor(
                                    s_sb_new[:, esl], s_sb_prev[:, esl],
                                    dtot[:, r:r + 1],
                                    sp[:, hh * PD:(hh + 1) * PD],
                                    ALU.mult, ALU.add)
                        nc.scalar.copy(s_bf_new[:, prows],
                                       s_sb_new[:, prows])

                        # v = (Y + D*x) * silu(z) -> token-pair staging tile
                        t1 = sc.tile([128, Q], FP32, tag="t1")
                        nc.vector.scalar_tensor_tensor(
                            t1[:], xbca[pi][:, cols], c_dp[:, pi:pi + 1],
                            yp[:], ALU.mult, ALU.add)
                        lq = pi * GSZ + (ci % 4) * Q
                        nc.vector.tensor_mul(vch[pi][:], t1[:],
                                             sz_of[ci // 4][:, lq:lq + Q])

                    j = ci % 8
                    a2a_dst = a2aE_in if ci < 8 else a2aO_in
                    for pi in range(4):
                        nc.sync.dma_start(
                            a2a_dst[512 * j + 128 * pi:
                                    512 * j + 128 * (pi + 1), :],
                            vch[pi][:])

                    st["sb"], st["sf"] = s_sb_new, s_bf_new

                # w_out quarter tiles [128, 512]: wkq[k][n]
                wkq = [[None] * 4 for _ in range(32)]
                wq_pools = {}

                def load_wq(k, n, ring):
                    pool = wq_pools["A" if n < 2
                                    else ("C" if n == 2 and k < 16 else "B")]
                    t = pool.tile([128, 512], BF16, name=f"wq{k}_{n}")
                    ring(t[:], w_out_t[k * 128:(k + 1) * 128,
                                      n * 512:(n + 1) * 512])
                    wkq[k][n] = t

                # ---- out_proj helpers (even-half n0/n1 interleaved in g3) --
                op_state = {}

                def emit_vth(h, a2a_o):
                    vth = fin.tile([128, 32 * 128], BF16, name=f"vth{h}")
                    for k in range(32):
                        ring = nc.sync if k % 2 == 0 else nc.scalar
                        ring.dma_start(vth[:, k * 128:(k + 1) * 128],
                                       a2a_o[k * 128:(k + 1) * 128, :])
                    op_state[f"vth{h}"] = vth

                def emit_ssp(h, pool_ss):
                    vth = op_state[f"vth{h}"]
                    sst = pool_ss.tile([128, 132], FP32, tag="ss",
                                       name=f"sst{h}")
                    ssp = sst[0:1, 0:128]
                    pst = sst[:, 128:129]
                    for k in range(32):
                        vsq = fin.tile([128, 128], BF16, tag="vsq", bufs=4,
                                       name=f"vsq{h}_{k}")
                        nc.scalar.activation(vsq[:],
                                             vth[:, k * 128:(k + 1) * 128],
                                             AF.Square)
                        nc.tensor.matmul(ssp[:], c_ones_col[:], vsq[:],
                                         start=(k == 0), stop=(k == 31))
                    ssr = fin.tile([1, 128], FP32, tag="ssr", name=f"ssr{h}")
                    nc.scalar.copy(ssr[:], ssp[:])
                    nc.tensor.transpose(pst[:], ssr[:], c_if32[0:1, 0:1])
                    gln = fin.tile([128, 1], FP32, tag="gln", name=f"gln{h}")
                    nc.scalar.activation(gln[:], pst[:], AF.Ln,
                                         scale=1.0 / (2 * DM), bias=c_eps[:])
                    gcol = fin.tile([128, 1], FP32, tag="gcol", name=f"gc{h}")
                    nc.scalar.activation(gcol[:], gln[:], AF.Exp, scale=-0.5)
                    op_state[f"gcol{h}"] = gcol

                def emit_op_burst(h, n, pool_op, k0, k1):
                    # partial accumulation burst: sized so the weight
                    # quarter-tiles have surely landed, avoiding in-order
                    # PE-queue head-of-line blocking of later scan chunks
                    key = f"ops{h}_{n}"
                    if k0 == 0:
                        op_state[key] = pool_op.tile([128, 512], FP32,
                                                     tag="op", name=key)
                    ops = op_state[key]
                    vth = op_state[f"vth{h}"]
                    for k in range(k0, k1):
                        nc.tensor.matmul(ops[:],
                                         vth[:, k * 128:(k + 1) * 128],
                                         wkq[k][n][:],
                                         start=(k == 0), stop=(k == 31))
                    if k1 < 32:
                        return
                    osb = fin.tile([128, 512], FP32, tag="osb", bufs=2,
                                   name=f"osb{h}_{n}")
                    nc.vector.tensor_scalar_mul(osb[:], ops[:],
                                                op_state[f"gcol{h}"])
                    nc.sync.dma_start(
                        out_s[h * 128:(h + 1) * 128,
                              n * 512:(n + 1) * 512], osb[:])

                def emit_op_pass(h, n, pool_op):
                    emit_op_burst(h, n, pool_op, 0, 32)

                # ---- schedule ----
                # group 0: dt tile first so dtprep/head chains start early;
                # conv per feat-tile right behind its m-tile
                emit_ip_loads(0)
                emit_ip_mtile(0, 10)
                for m in range(6):
                    emit_ip_mtile(0, m)
                    emit_conv(0, m, m + 1)
                emit_dtprep(0)
                for m in range(6, 10):
                    emit_ip_mtile(0, m)
                emit_silu_x(0)
                emit_silu_z(0)
                for ci in (0, 1):
                    emit_head_b(ci)
                # per-slot m-tile plan: dt (m10) first, z tails at j3
                MPLAN = [(10, 0, 1), (2, 3, 4), (5, 6, 7), (8, 9)]
                pool_op = pool_ss = None
                for g in range(4):
                    nxt = g + 1
                    if g == 3:
                        ip.release()
                        wpool.release()
                        ippsum.release()
                        cvt.release()
                        convp.release()
                        pool_op = tc.alloc_tile_pool(name="ps_op", bufs=2,
                                                     space="PSUM",
                                                     side="right")
                        pool_ss = tc.alloc_tile_pool(name="ps_ss", bufs=1,
                                                     space="PSUM",
                                                     side="right")
                        wq_pools["A"] = tc.alloc_tile_pool(
                            name="wqA", bufs=1, side="right")
                        for k in range(0, 32):
                            load_wq(k, 0, nc.scalar.dma_start if k % 2
                                    else nc.sync.dma_start)
                        for k in range(0, 16):
                            load_wq(k, 1, nc.scalar.dma_start if k % 2
                                    else nc.sync.dma_start)
                    if nxt < 4:
                        emit_ip_loads(nxt)
                    for j in range(4):
                        ci = 4 * g + j
                        if j in (1, 2):
                            emit_head_b(ci + 1)
                        emit_chunk(ci)
                        if ci == 7:
                            nc.gpsimd.collective_compute(
                                "AllToAll", ALU.bypass, replica_groups=rg,
                                ins=[a2aE_in.opt()], outs=[a2aE_out.opt()])
                        elif ci == 15:
                            nc.gpsimd.collective_compute(
                                "AllToAll", ALU.bypass, replica_groups=rg,
                                ins=[a2aO_in.opt()], outs=[a2aO_out.opt()])
                        if nxt < 4:
                            for m in MPLAN[j]:
                                emit_ip_mtile(nxt, m)
                            if j == 1:
                                emit_conv(nxt, 0, 3)
                                emit_dtprep(nxt)
                            elif j == 2:
                                emit_conv(nxt, 3, 6)
                                emit_silu_x(nxt)
                                emit_head_b(4 * nxt)
                                emit_head_b(4 * nxt + 1)
                            elif j == 3:
                                emit_silu_z(nxt)
                        if g == 2 and ci == 9:
                            emit_vth(0, a2aE_out)
                        if g == 3:
                            if j == 0:
                                emit_ssp(0, pool_ss)
                                for k in range(16, 32):
                                    load_wq(k, 1, nc.scalar.dma_start
                                            if k % 2 else nc.sync.dma_start)
                            elif j == 1:
                                emit_op_burst(0, 0, pool_op, 0, 16)
                                wq_pools["C"] = tc.alloc_tile_pool(
                                    name="wqC", bufs=1, side="right")
                                for k in range(0, 16):
                                    load_wq(k, 2, nc.scalar.dma_start)
                            elif j == 2:
                                emit_op_burst(0, 0, pool_op, 16, 32)
                                emit_op_burst(0, 1, pool_op, 0, 16)
                            elif j == 3:
                                emit_op_burst(0, 1, pool_op, 16, 32)

                # ---- tail: free mid-phase pools, stream n2/n3, finish ----
                bc.release()
                scst.release()
                sc.release()
                mid.release()
                wq_pools["B"] = tc.alloc_tile_pool(name="wqB", bufs=1)
                for k in range(16, 32):
                    load_wq(k, 2, nc.scalar.dma_start)
                for k in range(32):
                    load_wq(k, 3, nc.scalar.dma_start)
                emit_vth(1, a2aO_out)
                emit_op_pass(0, 2, pool_op)
                emit_op_pass(0, 3, pool_op)
                emit_ssp(1, pool_ss)
                for n in range(4):
                    emit_op_pass(1, n, pool_op)
                pool_ss.release()
                pool_op.release()
                wq_pools["B"].release()
                wq_pools["C"].release()
                wq_pools["A"].release()
                fin.release()
    nc.compile()
    return nc


def _get_built():
    global _BUILT
    if _BUILT is None:
        _BUILT = _build()
    return _BUILT


def kernel(**inputs):
    hs = np.ascontiguousarray(np.asarray(inputs["hidden_states"],
                                         dtype=np.float32))
    rd = np.ascontiguousarray(np.asarray(inputs["residual"], dtype=np.float32))
    B, L, Dm = hs.shape
    norm_w = np.asarray(inputs["norm_w"], dtype=np.float32)
    in_w = np.asarray(inputs["in_proj_w"], dtype=np.float32)
    conv_w = np.asarray(inputs["conv_w"], dtype=np.float32)
    conv_b = np.asarray(inputs["conv_b"], dtype=np.float32)
    A_log = np.asarray(inputs["A_log"], dtype=np.float32)
    D_param = np.asarray(inputs["D_param"], dtype=np.float32)
    dt_bias = np.asarray(inputs["dt_bias"], dtype=np.float32)
    gnw = np.asarray(inputs["gate_norm_w"], dtype=np.float32)
    out_w = np.asarray(inputs["out_proj_w"], dtype=np.float32)

    hid2 = hs.reshape(BT, DM)
    res2 = rd.reshape(BT, DM)
    hsum2 = hid2 + res2
    Wn = in_w * norm_w[None, :]
    Wg = out_w * gnw[None, :]
    w_out_t = np.ascontiguousarray(Wg.T).astype(ml_dtypes.bfloat16)

    sidx = np.arange(128)[:, None]
    tidx = np.arange(128)[None, :]
    m0 = np.where(sidx > tidx, np.float32(0.0), np.float32(1.0))

    common = {
        "w_out_t": w_out_t,
        "hr": hsum2[0:1024].astype(ml_dtypes.bfloat16),
        "ones_col_bf": np.ones((128, 1), ml_dtypes.bfloat16),
        "m0_bf": m0.astype(ml_dtypes.bfloat16),
        "i_bf": np.eye(128, dtype=ml_dtypes.bfloat16),
        "i_f32": np.eye(128, dtype=np.float32),
    }

    in_maps = []
    for c in range(NCORES):
        rows = np.r_[4096 + 512 * c:4096 + 512 * (c + 1),
                     8192 + 128 * c:8192 + 128 * (c + 1),
                     9216 + 128 * c:9216 + 128 * (c + 1),
                     512 * c:512 * (c + 1),
                     10240 + 8 * c:10240 + 8 * (c + 1)]
        w_in_t = np.ascontiguousarray(Wn[rows, :].T).astype(ml_dtypes.bfloat16)
        crows = np.r_[512 * c:512 * (c + 1),
                      4096 + 128 * c:4096 + 128 * (c + 1),
                      5120 + 128 * c:5120 + 128 * (c + 1)]
        orows = np.concatenate(
            [np.arange(512 * j + 64 * c, 512 * j + 64 * c + 64)
             for j in (2, 3)])
        in_maps.append(dict(
            common,
            hso=hsum2[orows].astype(ml_dtypes.bfloat16),
            w_in_t=w_in_t,
            a_col=(-np.exp(A_log[8 * c:8 * (c + 1)])).reshape(8, 1)
                  .astype(np.float32),
            dtb_col=dt_bias[8 * c:8 * (c + 1)].reshape(8, 1).astype(np.float32),
            dp_col=np.ascontiguousarray(
                np.repeat(D_param[8 * c:8 * (c + 1)], PD).reshape(4, 128).T)
                .astype(np.float32),
            convw=np.ascontiguousarray(
                conv_w[crows, :].reshape(6, 128, 4).transpose(1, 0, 2)
                .reshape(128, 24)).astype(np.float32),
            convb=np.ascontiguousarray(
                conv_b[crows].reshape(6, 128).T).astype(np.float32),
        ))

    nc = _get_built()
    res_k = bass_utils.run_bass_kernel_spmd(
        nc, in_maps, core_ids=list(range(NCORES)))
    global LAST_RESULTS
    LAST_RESULTS = res_k

    out = np.empty((BT, DM), np.float32)
    for c in range(NCORES):
        for h in range(2):
            dst = slice(1024 * h + 128 * c, 1024 * h + 128 * (c + 1))
            src_r = slice(128 * h, 128 * (h + 1))
            out[dst, :] = res_k.results[c]["out_s"][src_r]
    return out.reshape(B, L, Dm), hsum2.reshape(B, L, Dm)
